# revision 28
# baseline (speedup 1.0000x reference)
"""Trainium2 Bass kernel for nn_DenseCRFFDR (dense CRF mean-field, 2 labels).

Self-contained: hardcodes shapes/sharding. kernel(**inputs) takes FULL
numpy inputs and returns the FULL output tuple (h_new, f_1).

Algorithm (validated in numpy): with 2 labels, softmax == sigmoid(diff) and
q1 = 1-q0, so the whole 5-iteration CRF collapses to a recursion on a single
map:  diff <- Cd - ad*blur3d(q0) - bd*rden*num_raw(q0),  q0 = sigmoid(diff).
All 2x2 matrix algebra folds into host scalars (ad, bd, A1, B1); the
bilateral's H-axis (partition) shifts and all 27-term accumulations run on
the TensorEngine as shift-matrix / banded matmuls accumulating in PSUM.
The DVE only does the 27 Hadamard products per iteration (fp16, 2x mode).
rden factors out of the bilateral sum entirely; the blur-of-ones (G1) and
bilateral-of-ones (W1 = den - maskW1*E) constants are host-side geometry,
so no ones-pass runs on device.

Sharding: D=64 split across 8 cores (8 planes each + 2-plane halo); per-
iteration halo exchange of q0 via AllGather of boundary planes, neighbor
slices addressed dynamically with partition_id; edge cores mask wrapped data.
"""
import math
import numpy as np

# ---------------- problem constants (hardcoded per contract) ----------------
D, H, W = 64, 128, 128
N_CORES = 8
DL = D // N_CORES            # 8 own planes per core
HALO = 2
NP = DL + 2 * HALO           # 12 plane slots
WS = 136                     # stored width; valid cols [4, 132)
CV = 4                       # first valid col
NUM_ITERS = 5
BLUR_RADIUS = 2
S2R = 2.0 * 0.5 ** 2         # 2*sigma_range^2 = 0.5
S2S = 2.0 * 1.5 ** 2         # 2*sigma_spatial^2 = 4.5
LOG_SQRT_2PI = 0.5 * float(np.log(2.0 * np.pi))

_offs = np.arange(-BLUR_RADIUS, BLUR_RADIUS + 1)
_k1d = np.exp(-_offs.astype(np.float64) ** 2 / 2.0)
K1D = (_k1d / _k1d.sum()).astype(np.float32)

OFFSETS = [(dz, dy, dx) for dz in (-1, 0, 1) for dy in (-1, 0, 1)
           for dx in (-1, 0, 1)]
# mirrored dy=0 offsets are derived from their negatives via shifted reads
MIRRORED = [s for s in OFFSETS if s[1] == 0 and s < (0, 0, 0)]
COMPUTED = [s for s in OFFSETS if not (s[1] == 0 and s < (0, 0, 0))]
# sources of mirrors need the extended 10-plane domain
MSRC = {(-dz, 0, -dx) for (dz, dy, dx) in MIRRORED}

N_GP_PRODUCTS = 0   # products per iteration offloaded to GPSIMD

DEBUG = 0

_CACHE = {}


# ---------------- host-side helpers ----------------
def _gmat(s):
    dz, dy, dx = s
    return math.exp(-(dz * dz + dy * dy + dx * dx) / S2S)


def _shift_mat(dy, val=1.0):
    # (S @ v)[m] = val * v[m+dy]
    S = np.zeros((H, H), np.float32)
    for m in range(H):
        k = m + dy
        if 0 <= k < H:
            S[m, k] = val
    return S


def _banded_bh(scale=1.0):
    Bh = np.zeros((H, H), np.float32)
    for m in range(H):
        for t in range(-BLUR_RADIUS, BLUR_RADIUS + 1):
            k = m + t
            if 0 <= k < H:
                Bh[m, k] = scale * K1D[t + BLUR_RADIUS]
    return Bh


def _host_scalars(spatial_w, bilateral_w, compat):
    CS = (compat @ spatial_w).astype(np.float64)
    CB = (compat @ bilateral_w).astype(np.float64)
    A = CS[0] - CS[1]
    B = CB[0] - CB[1]
    return float(A[1]), float(B[1]), float(A[0] - A[1]), float(B[0] - B[1])


def _build_mats16(ad):
    """fp16 stationaries (transposed for lhsT), order:
    0: I, 1: SD-1, 2: SD+1, 3..7: cI taps dx=-2..2, 8..12: DH taps dz=-2..2."""
    mats = [np.eye(H, dtype=np.float32), _shift_mat(-1), _shift_mat(+1)]
    for t in range(5):
        mats.append(K1D[t] * np.eye(H, dtype=np.float32))
    for t in range(5):
        mats.append(_banded_bh(-ad * K1D[t]))
    return np.stack([m.T for m in mats]).astype(np.float16)


M_I, M_SDM, M_SDP = 0, 1, 2
M_CI = 3      # +t
M_DH = 8      # +t
N_MATS = 13


def _vslab(x, base, lo, hi, dtype):
    n = hi - lo
    out = np.zeros((H, n, WS), dtype)
    for i, d in enumerate(range(base + lo, base + hi)):
        if 0 <= d < D:
            out[:, i, CV:CV + W] = x[d].astype(dtype)
    return out


def _yshift(t, dy):
    out = np.zeros_like(t)
    if dy == 0:
        out[:] = t
    elif dy == 1:
        out[1:] = t[:-1]
    else:
        out[:-1] = t[1:]
    return out


def _xshift1(t):
    out = np.zeros_like(t)
    out[:, :, :-1] = t[:, :, 1:]
    return out


def _g1_full():
    if "g1" not in _CACHE:
        x = np.ones((D, H, W), np.float64)
        k = K1D.astype(np.float64)
        for ax in (0, 1, 2):
            acc = np.zeros_like(x)
            for t in range(-BLUR_RADIUS, BLUR_RADIUS + 1):
                sl_src = [slice(None)] * 3
                sl_dst = [slice(None)] * 3
                n = x.shape[ax]
                if t >= 0:
                    sl_dst[ax] = slice(t, n)
                    sl_src[ax] = slice(0, n - t)
                else:
                    sl_dst[ax] = slice(0, n + t)
                    sl_src[ax] = slice(-t, n)
                acc[tuple(sl_dst)] += k[t + BLUR_RADIUS] * x[tuple(sl_src)]
            x = acc
        _CACHE["g1"] = x.astype(np.float32)
    return _CACHE["g1"]


def _build_core_inputs(image, h, f_1, w_0, A1, B1, ad, bd):
    I = np.asarray(image[0, 0], np.float32)
    h0 = np.asarray(h[0, 0], np.float32)
    h1 = np.asarray(h[0, 1], np.float32)
    f = np.asarray(f_1[0, 0], np.float32)
    w0 = float(np.asarray(w_0)[0])

    logprob = -0.5 * I * I - LOG_SQRT_2PI
    Ud_full = (h0 - h1) * (-(w0 + logprob - f))
    hd = (h0 - h1).astype(np.float32)
    q0_full = (1.0 / (1.0 + np.exp(-hd))).astype(np.float32)
    G1 = _g1_full()
    CdBase_full = Ud_full - A1 * G1 - B1          # (D,H,W) fp32

    gsum = sum(math.exp(-(dz * dz + 1 + dx * dx) / S2S)
               for dz in (-1, 0, 1) for dx in (-1, 0, 1))
    gvec = np.zeros((H, 1), np.float32)
    gvec[0, 0] = gsum
    gvec[H - 1, 0] = gsum

    lng = np.zeros((H, 4), np.float32)
    for d2 in range(4):
        lng[:, d2] = -d2 / S2S

    bdvec = np.full((H, 1), -bd, np.float32)
    mats16 = _build_mats16(ad)

    in_maps = []
    for c in range(N_CORES):
        base = c * DL
        img_y0 = _vslab(I, base, -HALO, DL + HALO, np.float16)
        q0i = _vslab(q0_full, base, -HALO, DL + HALO, np.float16)
        mask = np.zeros((H, 4, WS), np.float16)
        if c > 0:
            mask[:, 0:2, CV:CV + W] = 1.0
        if c < N_CORES - 1:
            mask[:, 2:4, CV:CV + W] = 1.0
        # maskW1: sum of g_s over offsets with x+s outside the volume
        mw1 = np.zeros((H, DL, W), np.float32)
        for (dz, dy, dx) in OFFSETS:
            g = _gmat((dz, dy, dx))
            oob = np.zeros((H, DL, W), bool)
            for i in range(DL):
                if not (0 <= base + i + dz < D):
                    oob[:, i, :] = True
            if dy == -1:
                oob[0, :, :] = True
            elif dy == 1:
                oob[H - 1, :, :] = True
            if dx == -1:
                oob[:, :, 0] |= True
            elif dx == 1:
                oob[:, :, W - 1] |= True
            mw1 += g * oob
        in_maps.append({
            "mats16": mats16,
            "img_y0": img_y0,
            "img_ym": _yshift(img_y0, -1),
            "img_yp": _yshift(img_y0, +1),
            "img_x1": _xshift1(img_y0),
            "cdbase": np.ascontiguousarray(
                np.stack([CdBase_full[base + i] for i in range(DL)], 1)),
            "mw1e": (B1 * mw1).astype(np.float16),
            "q0i": q0i,
            "q1i": _xshift1(q0i),
            "mask": mask,
            "gvec": gvec,
            "lng": lng,
            "bdvec": bdvec,
        })
    return in_maps


# ---------------- bass program ----------------
def _build_nc(stage="full"):
    import concourse.bass as bass
    import concourse.tile as tile
    from concourse import bacc, mybir
    from contextlib import ExitStack

    dt16 = mybir.dt.float16
    dt32 = mybir.dt.float32
    AF = mybir.ActivationFunctionType
    OP = mybir.AluOpType

    nc = bacc.Bacc("TRN2", target_bir_lowering=False, debug=False,
                   num_devices=N_CORES)

    din = {}
    din["mats16"] = nc.dram_tensor("mats16", [N_MATS, H, H], dt16,
                                   kind="ExternalInput").ap()
    for nm in ("img_y0", "img_ym", "img_yp", "img_x1", "q0i", "q1i"):
        din[nm] = nc.dram_tensor(nm, [H, NP, WS], dt16,
                                 kind="ExternalInput").ap()
    din["cdbase"] = nc.dram_tensor("cdbase", [H, DL, W], dt32,
                                   kind="ExternalInput").ap()
    din["mw1e"] = nc.dram_tensor("mw1e", [H, DL, W], dt16,
                                 kind="ExternalInput").ap()
    din["mask"] = nc.dram_tensor("mask", [H, 4, WS], dt16,
                                 kind="ExternalInput").ap()
    din["gvec"] = nc.dram_tensor("gvec", [H, 1], dt32,
                                 kind="ExternalInput").ap()
    din["lng"] = nc.dram_tensor("lng", [H, 4], dt32,
                                kind="ExternalInput").ap()
    din["bdvec"] = nc.dram_tensor("bdvec", [H, 1], dt32,
                                  kind="ExternalInput").ap()
    h_out = nc.dram_tensor("h_out", [2, DL, H, W], dt32,
                           kind="ExternalOutput").ap()
    dbg = {}
    if DEBUG:
        for nm, shp, dt_ in [("dbg_rho", [H, 10, 132], dt16),
                             ("dbg_den", [H, DL, W], dt32),
                             ("dbg_rden", [H, DL, W], dt16),
                             ("dbg_cd", [H, DL, W], dt32),
                             ("dbg_diff1", [H, DL, W], dt32)]:
            dbg[nm] = nc.dram_tensor(nm, shp, dt_, kind="ExternalOutput").ap()

    with tile.TileContext(nc) as tc, ExitStack() as ctx:
        consts = ctx.enter_context(tc.tile_pool(name="consts", bufs=1))
        work = ctx.enter_context(tc.tile_pool(name="work", bufs=1))
        rhop = ctx.enter_context(tc.tile_pool(name="rhop", bufs=1))
        vpool = ctx.enter_context(tc.tile_pool(name="vpool", bufs=6))
        tmp = ctx.enter_context(tc.tile_pool(name="tmp", bufs=6))
        tmp2 = ctx.enter_context(tc.tile_pool(name="tmp2", bufs=2))
        psum = ctx.enter_context(
            tc.tile_pool(name="psum", bufs=2, space="PSUM"))
        psum_t1 = ctx.enter_context(
            tc.tile_pool(name="psum_t1", bufs=1, space="PSUM"))
        dram = ctx.enter_context(tc.tile_pool(name="dram", bufs=2,
                                              space="DRAM"))

        # ---- image slabs first: the rho chains depend on them ----
        imgs = ctx.enter_context(tc.tile_pool(name="imgs", bufs=1))
        img = {}
        for nm in ("img_y0", "img_x1", "img_ym", "img_yp"):
            t = imgs.tile([H, NP, WS], dt16, name=nm)
            nc.sync.dma_start(t[:], din[nm][:])
            img[nm] = t
        img_y = {-1: img["img_ym"], 0: img["img_y0"], 1: img["img_yp"]}

        # ---- constants ----
        lngt = consts.tile([H, 4], dt32)
        nc.sync.dma_start(lngt[:], din["lng"][:])
        gvect = consts.tile([H, 1], dt32)
        nc.sync.dma_start(gvect[:], din["gvec"][:])
        mats = consts.tile([H, N_MATS, H], dt16)
        for i in range(N_MATS):
            nc.sync.dma_start(mats[:, i, :], din["mats16"][i, :, :])
        bdvt = consts.tile([H, 1], dt32)
        nc.sync.dma_start(bdvt[:], din["bdvec"][:])
        cdb32 = consts.tile([H, DL, W], dt32)
        nc.sync.dma_start(cdb32[:], din["cdbase"][:])
        mw1e = consts.tile([H, DL, W], dt16)
        nc.sync.dma_start(mw1e[:], din["mw1e"][:])
        maskt = consts.tile([H, 4, WS], dt16)
        nc.sync.dma_start(maskt[:], din["mask"][:])

        def mat(i):
            return mats[:, i, :]

        # ---- persistent work tiles ----
        q0 = work.tile([H, NP, WS], dt16)
        q1 = work.tile([H, NP, WS], dt16)
        nc.sync.dma_start(q0[:], din["q0i"][:])
        nc.sync.dma_start(q1[:], din["q1i"][:])
        t1sb = work.tile([H, NP, W], dt16)
        cdhi = work.tile([H, DL, W], dt16)
        cdlo = work.tile([H, DL, W], dt16)
        rden = work.tile([H, DL, W], dt16)
        dtile = work.tile([H, DL, W], dt32)   # diff (combine output)

        # =========== PHASE 1: rho', den, rden, Cd ===========

        # rho tiles: mirror sources on 10 plane slots, others on 8
        rho = {}
        for s in COMPUTED:
            dz, dy, dx = s
            big = s in MSRC
            lo = 1 if big else 2    # first covered frame slot
            n = 10 if big else 8
            ia = img_y[dy][:, lo:lo + n, 2:134]
            if dx % 2 == 0:
                ib = img["img_y0"][:, lo + dz:lo + n + dz, 2 + dx:134 + dx]
            else:
                ib = img["img_x1"][:, lo + dz:lo + n + dz, 1 + dx:133 + dx]
            dlt = tmp.tile([H, 10, 132], dt16, tag="delta")
            nc.vector.tensor_tensor(dlt[:, 0:n, :], ia, ib, OP.subtract)
            sq = tmp.tile([H, 10, 132], dt16, tag="sq")
            nc.vector.tensor_tensor(sq[:, 0:n, :], dlt[:, 0:n, :],
                                    dlt[:, 0:n, :], OP.mult)
            r = rhop.tile([H, n, 132], dt16, name=f"rho_{s}")
            d2 = dz * dz + dy * dy + dx * dx
            nc.scalar.activation(r[:], sq[:, 0:n, :], AF.Exp,
                                 scale=-1.0 / S2R,
                                 bias=lngt[:, d2:d2 + 1])
            # (tile, own-plane index offset within tile, col shift)
            rho[s] = (r, 1 - lo, 0)
        for s in MIRRORED:
            dz, dy, dx = s
            src, po, _ = rho[(-dz, 0, -dx)]
            rho[s] = (src, po + dz, dx)

        # den psum: sum_s SD[dy] @ rho_s (own planes)
        ps_den = psum.tile([H, DL, W], dt32, tag="ps")
        for si, s in enumerate(OFFSETS):
            dz, dy, dx = s
            r, po, px = rho[s]
            sd = {-1: M_SDM, 0: M_I, 1: M_SDP}[dy]
            for hf in range(2):
                rhs = r[:, 1 + po + hf * 4: 5 + po + hf * 4, 2 + px: 130 + px]
                nc.tensor.matmul(ps_den[:, hf * 4:hf * 4 + 4, :],
                                 mat(sd), rhs,
                                 start=(si == 0), stop=(si == 26))

        # E = exp(-I^2/S2R); den += gvec*E; rden = 1/den (fast recip + Newton)
        isq = tmp2.tile([H, DL, W], dt16, tag="scr8")
        nc.vector.tensor_tensor(isq[:],
                                img["img_y0"][:, 2:10, 4:132],
                                img["img_y0"][:, 2:10, 4:132], OP.mult)
        emap = work.tile([H, DL, W], dt16)
        nc.scalar.activation(emap[:], isq[:], AF.Exp, scale=-1.0 / S2R)
        # tA = (B1*maskW1)*E  (independent of den; hoisted off the tail)
        tA = tmp2.tile([H, DL, W], dt16, tag="scr8")
        nc.vector.tensor_tensor(tA[:], mw1e[:], emap[:], OP.mult)
        den_sb = work.tile([H, DL, W], dt32)
        nc.vector.scalar_tensor_tensor(den_sb[:], emap[:], gvect[:],
                                       ps_den[:], OP.mult, OP.add)
        r32 = tmp2.tile([H, DL, W], dt32, tag="u32")
        rscr = tmp2.tile([H, DL, W], dt32, tag="u32")
        nc.vector.reciprocal_approx_accurate(r32[:], den_sb[:], rscr[:])
        nc.scalar.copy(rden[:], r32[:])

        # Cd = CdBase + tA * rden ; split into fp16 hi+lo
        tB = tmp2.tile([H, DL, W], dt16, tag="scr8")
        nc.vector.tensor_tensor(tB[:], tA[:], rden[:], OP.mult)
        cd32 = work.tile([H, DL, W], dt32)
        nc.vector.tensor_tensor(cd32[:], tB[:], cdb32[:], OP.add)
        nc.vector.tensor_copy(cdhi[:], cd32[:])
        with nc.allow_low_precision(reason="hi/lo residual split"):
            nc.vector.scalar_tensor_tensor(cdlo[:], cdhi[:], -1.0,
                                           cd32[:], OP.mult, OP.add)

        if DEBUG:
            nc.sync.dma_start(dbg["dbg_rho"][:, 0:10, :],
                              rho[(0, 0, 1)][0][:])
            nc.sync.dma_start(dbg["dbg_den"][:], den_sb[:])
            nc.sync.dma_start(dbg["dbg_rden"][:], rden[:])
            nc.sync.dma_start(dbg["dbg_cd"][:], cd32[:])

        done = False
        if stage == "pre":
            dummy = work.tile([H, W], dt32)
            nc.vector.tensor_copy(dummy[:], cd32[:, 0, :])
            nc.sync.dma_start(h_out[0, 0, :, :], dummy[:])
            done = True

        # ======== one CRF message pass ========
        def message_pass(psA, psB, qa, qb):
            # psA = Cd (fp16 hi+lo) + blur terms (-ad folded in DH mats)
            for hf in range(2):
                nc.tensor.matmul(psA[:, hf * 4:hf * 4 + 4, :], mat(M_I),
                                 cdhi[:, hf * 4:hf * 4 + 4, :],
                                 start=True, stop=False)
                nc.tensor.matmul(psA[:, hf * 4:hf * 4 + 4, :], mat(M_I),
                                 cdlo[:, hf * 4:hf * 4 + 4, :],
                                 start=False, stop=False)
            t1ps = psum_t1.tile([H, NP, W], dt32, tag="t1")
            for ti, dx in enumerate(range(-2, 3)):
                for third in range(3):
                    sl = slice(third * 4, third * 4 + 4)
                    if dx % 2 == 0:
                        rhs = qa[:, sl, CV + dx:CV + W + dx]
                    else:
                        rhs = qb[:, sl, CV - 1 + dx:CV - 1 + W + dx]
                    nc.tensor.matmul(t1ps[:, sl, :], mat(M_CI + ti), rhs,
                                     start=(ti == 0), stop=(ti == 4))
            nc.scalar.copy(t1sb[:], t1ps[:])
            for ti, dz in enumerate(range(-2, 3)):
                for hf in range(2):
                    rhs = t1sb[:, 2 + hf * 4 + dz:6 + hf * 4 + dz, :]
                    nc.tensor.matmul(psA[:, hf * 4:hf * 4 + 4, :],
                                     mat(M_DH + ti), rhs,
                                     start=False, stop=(ti == 4 and hf == 1))
            # psB = raw bilateral sum (products on DVE/GPSIMD, adds on PE)
            # halo-independent (dz=0) and q0-only (even dx) products first:
            # they can run while the halo AllGather and q1 sigmoid complete
            ordered = sorted(OFFSETS, key=lambda s: (s[0] != 0, s[2] % 2 != 0))
            for si, s in enumerate(ordered):
                dz, dy, dx = s
                r, po, px = rho[s]
                exs = dx + px   # effective q shift incl. mirror col shift? no:
                # q shift is dx; rho col shift is px (mirror). They are
                # independent APs; alignment of q read depends on dx only.
                if dx % 2 == 0:
                    qr = qa[:, 2 + dz:10 + dz, CV + dx:CV + W + dx]
                else:
                    qr = qb[:, 2 + dz:10 + dz, CV - 1 + dx:CV - 1 + W + dx]
                rr = r[:, 1 + po:9 + po, 2 + px:130 + px]
                v = vpool.tile([H, DL, W], dt16, tag="v")
                eng = (nc.gpsimd
                       if 9 <= si < 9 + N_GP_PRODUCTS else nc.vector)
                eng.tensor_tensor(v[:], qr, rr, OP.mult)
                sd = {-1: M_SDM, 0: M_I, 1: M_SDP}[dy]
                for hf in range(2):
                    nc.tensor.matmul(psB[:, hf * 4:hf * 4 + 4, :], mat(sd),
                                     v[:, hf * 4:hf * 4 + 4, :],
                                     start=(si == 0), stop=(si == 26 and
                                                            hf == 1))

        # =========== PHASE 3: iterations ===========
        if done:
            n_iters = 0
        else:
            if stage.startswith("it"):
                n_iters = int(stage[2:])
            elif stage.startswith("noex") and len(stage) > 4:
                n_iters = int(stage[4:])
            elif stage.startswith("nag"):
                n_iters = int(stage[3:])
            else:
                n_iters = NUM_ITERS
            rank = nc.partition_id()
            lo_r = (rank + N_CORES - 1) % N_CORES
            hi_r = (rank + 1) % N_CORES

        for it in range(n_iters):
            if it > 0:
                for hf in range(2):
                    sl = slice(2 + hf * 4, 6 + hf * 4)
                    dsl = dtile[:, hf * 4:hf * 4 + 4, :]
                    nc.scalar.activation(q0[:, sl, CV:CV + W], dsl,
                                         AF.Sigmoid)
                    nc.scalar.activation(q1[:, sl, CV - 1:CV - 1 + W], dsl,
                                         AF.Sigmoid)
                if not stage.startswith("noex"):
                    cc_in = dram.tile([H, 4, WS], dt16, tag="cc_in")
                    cc_out = dram.tile([N_CORES, H, 4, WS], dt16,
                                       addr_space="Shared", tag="cc_out")
                    nc.sync.dma_start(cc_in[:, 0:2, :], q0[:, 2:4, :])
                    nc.sync.dma_start(cc_in[:, 2:4, :], q0[:, 8:10, :])
                    if not stage.startswith("nag"):
                        nc.gpsimd.collective_compute(
                            "AllGather", OP.bypass,
                            replica_groups=[list(range(N_CORES))],
                            ins=[cc_in[:].opt()], outs=[cc_out[:].opt()])
                    stg = tmp2.tile([H, 4, WS], dt16, tag="stage")
                    nc.sync.dma_start(stg[:, 0:2, :],
                                      cc_out[bass.ds(lo_r, 1), :, 2:4, :])
                    nc.sync.dma_start(stg[:, 2:4, :],
                                      cc_out[bass.ds(hi_r, 1), :, 0:2, :])
                    nc.vector.tensor_tensor(q0[:, 0:2, :], stg[:, 0:2, :],
                                            maskt[:, 0:2, :], OP.mult)
                    nc.vector.tensor_tensor(q0[:, 10:12, :], stg[:, 2:4, :],
                                            maskt[:, 2:4, :], OP.mult)
                    nc.vector.tensor_tensor(q1[:, 0:2, 0:WS - 1],
                                            stg[:, 0:2, 1:WS],
                                            maskt[:, 0:2, 1:WS], OP.mult)
                    nc.vector.tensor_tensor(q1[:, 10:12, 0:WS - 1],
                                            stg[:, 2:4, 1:WS],
                                            maskt[:, 2:4, 1:WS], OP.mult)
            psA = psum.tile([H, DL, W], dt32, tag="ps")
            psB = psum.tile([H, DL, W], dt32, tag="ps")
            message_pass(psA, psB, q0, q1)
            # combine per half: dtile = psA + (-bd)*rden*psB
            for hf in range(2):
                hsl = slice(hf * 4, hf * 4 + 4)
                u32 = tmp2.tile([H, 4, W], dt32, tag="u32h")
                nc.vector.scalar_tensor_tensor(u32[:], psB[:, hsl, :],
                                               bdvt[:], rden[:, hsl, :],
                                               OP.mult, OP.mult)
                nc.vector.tensor_tensor(dtile[:, hsl, :], psA[:, hsl, :],
                                        u32[:], OP.add)
            if DEBUG and it == 0:
                nc.sync.dma_start(dbg["dbg_diff1"][:], dtile[:])

        # =========== output ===========
        if not done:
            h0t = work.tile([H, DL, W], dt32)
            nc.scalar.activation(h0t[:], dtile[:], AF.Sigmoid)
            h1t = work.tile([H, DL, W], dt32)
            nc.vector.tensor_scalar(h1t[:], h0t[:], -1.0, 1.0, OP.mult,
                                    OP.add)
            for d_ in range(DL):
                nc.sync.dma_start(h_out[0, d_, :, :], h0t[:, d_, :])
                nc.sync.dma_start(h_out[1, d_, :, :], h1t[:, d_, :])

    nc.compile()
    return nc


def _get_nc(stage="full"):
    key = f"nc_{stage}"
    if key not in _CACHE:
        _CACHE[key] = _build_nc(stage)
    return _CACHE[key]


def kernel(image, h, f_1, w_0, spatial_ker_weights, bilateral_ker_weights,
           compatibility_matrix):
    from concourse.bass_utils import run_bass_kernel_spmd

    A1, B1, ad, bd = _host_scalars(
        np.asarray(spatial_ker_weights, np.float64),
        np.asarray(bilateral_ker_weights, np.float64),
        np.asarray(compatibility_matrix, np.float64))
    in_maps = _build_core_inputs(np.asarray(image), np.asarray(h),
                                 np.asarray(f_1), np.asarray(w_0),
                                 A1, B1, ad, bd)
    nc = _get_nc()
    res = run_bass_kernel_spmd(nc, in_maps, list(range(N_CORES)))
    _CACHE["last_results"] = res

    h_new = np.empty((1, 2, D, H, W), np.float32)
    for c in range(N_CORES):
        out = res.results[c]["h_out"]          # [2, DL, H, W]
        h_new[0, :, c * DL:(c + 1) * DL] = out
    return (h_new, np.asarray(f_1))


# revision 29
# speedup vs baseline: 1.2153x; 1.2153x over previous
"""Trainium2 Bass kernel for nn_DenseCRFFDR (dense CRF mean-field, 2 labels).

Self-contained: hardcodes shapes/sharding. kernel(**inputs) takes FULL
numpy inputs and returns the FULL output tuple (h_new, f_1).

Algorithm (validated in numpy): with 2 labels, softmax == sigmoid(diff) and
q1 = 1-q0, so the whole 5-iteration CRF collapses to a recursion on a single
map:  diff <- Cd - ad*blur3d(q0) - bd*rden*num_raw(q0),  q0 = sigmoid(diff).
All 2x2 matrix algebra folds into host scalars (ad, bd, A1, B1); the
bilateral's H-axis (partition) shifts and all 27-term accumulations run on
the TensorEngine as shift-matrix / banded matmuls accumulating in PSUM.
The DVE only does the 27 Hadamard products per iteration (fp16, 2x mode).
rden factors out of the bilateral sum entirely; the blur-of-ones (G1) and
bilateral-of-ones (W1 = den - maskW1*E) constants are host-side geometry,
so no ones-pass runs on device.

Sharding: D=64 split across 8 cores (8 planes each + 2-plane halo); per-
iteration halo exchange of q0 via AllGather of boundary planes, neighbor
slices addressed dynamically with partition_id; edge cores mask wrapped data.
"""
import math
import numpy as np

# ---------------- problem constants (hardcoded per contract) ----------------
D, H, W = 64, 128, 128
N_CORES = 8
DL = D // N_CORES            # 8 own planes per core
HALO = 2
NP = DL + 2 * HALO           # 12 plane slots
WS = 136                     # stored width; valid cols [4, 132)
CV = 4                       # first valid col
NUM_ITERS = 5
BLUR_RADIUS = 2
S2R = 2.0 * 0.5 ** 2         # 2*sigma_range^2 = 0.5
S2S = 2.0 * 1.5 ** 2         # 2*sigma_spatial^2 = 4.5
LOG_SQRT_2PI = 0.5 * float(np.log(2.0 * np.pi))

_offs = np.arange(-BLUR_RADIUS, BLUR_RADIUS + 1)
_k1d = np.exp(-_offs.astype(np.float64) ** 2 / 2.0)
K1D = (_k1d / _k1d.sum()).astype(np.float32)

OFFSETS = [(dz, dy, dx) for dz in (-1, 0, 1) for dy in (-1, 0, 1)
           for dx in (-1, 0, 1)]
# mirrored dy=0 offsets are derived from their negatives via shifted reads
MIRRORED = [s for s in OFFSETS if s[1] == 0 and s < (0, 0, 0)]
COMPUTED = [s for s in OFFSETS
            if not (s[1] == 0 and s < (0, 0, 0)) and s != (0, 0, 0)]
# sources of mirrors need the extended 10-plane domain
MSRC = {(-dz, 0, -dx) for (dz, dy, dx) in MIRRORED}

N_GP_PRODUCTS = 0   # products per iteration offloaded to GPSIMD

DEBUG = 0

_CACHE = {}


# ---------------- host-side helpers ----------------
def _gmat(s):
    dz, dy, dx = s
    return math.exp(-(dz * dz + dy * dy + dx * dx) / S2S)


def _shift_mat(dy, val=1.0):
    # (S @ v)[m] = val * v[m+dy]
    S = np.zeros((H, H), np.float32)
    for m in range(H):
        k = m + dy
        if 0 <= k < H:
            S[m, k] = val
    return S


def _banded_bh(scale=1.0):
    Bh = np.zeros((H, H), np.float32)
    for m in range(H):
        for t in range(-BLUR_RADIUS, BLUR_RADIUS + 1):
            k = m + t
            if 0 <= k < H:
                Bh[m, k] = scale * K1D[t + BLUR_RADIUS]
    return Bh


def _host_scalars(spatial_w, bilateral_w, compat):
    CS = (compat @ spatial_w).astype(np.float64)
    CB = (compat @ bilateral_w).astype(np.float64)
    A = CS[0] - CS[1]
    B = CB[0] - CB[1]
    return float(A[1]), float(B[1]), float(A[0] - A[1]), float(B[0] - B[1])


def _build_mats16(ad):
    """fp16 stationaries (transposed for lhsT), order:
    0: I, 1: SD-1, 2: SD+1, 3..7: cI taps dx=-2..2, 8..12: DH taps dz=-2..2."""
    mats = [np.eye(H, dtype=np.float32), _shift_mat(-1), _shift_mat(+1)]
    for t in range(5):
        mats.append(K1D[t] * np.eye(H, dtype=np.float32))
    for t in range(5):
        mats.append(_banded_bh(-ad * K1D[t]))
    return np.stack([m.T for m in mats]).astype(np.float16)


M_I, M_SDM, M_SDP = 0, 1, 2
M_CI = 3      # +t
M_DH = 8      # +t
N_MATS = 13


def _vslab(x, base, lo, hi, dtype):
    n = hi - lo
    out = np.zeros((H, n, WS), dtype)
    for i, d in enumerate(range(base + lo, base + hi)):
        if 0 <= d < D:
            out[:, i, CV:CV + W] = x[d].astype(dtype)
    return out


def _yshift(t, dy):
    out = np.zeros_like(t)
    if dy == 0:
        out[:] = t
    elif dy == 1:
        out[1:] = t[:-1]
    else:
        out[:-1] = t[1:]
    return out


def _xshift1(t):
    out = np.zeros_like(t)
    out[:, :, :-1] = t[:, :, 1:]
    return out


def _g1_full():
    if "g1" not in _CACHE:
        x = np.ones((D, H, W), np.float64)
        k = K1D.astype(np.float64)
        for ax in (0, 1, 2):
            acc = np.zeros_like(x)
            for t in range(-BLUR_RADIUS, BLUR_RADIUS + 1):
                sl_src = [slice(None)] * 3
                sl_dst = [slice(None)] * 3
                n = x.shape[ax]
                if t >= 0:
                    sl_dst[ax] = slice(t, n)
                    sl_src[ax] = slice(0, n - t)
                else:
                    sl_dst[ax] = slice(0, n + t)
                    sl_src[ax] = slice(-t, n)
                acc[tuple(sl_dst)] += k[t + BLUR_RADIUS] * x[tuple(sl_src)]
            x = acc
        _CACHE["g1"] = x.astype(np.float32)
    return _CACHE["g1"]


def _build_core_inputs(image, h, f_1, w_0, A1, B1, ad, bd):
    I = np.asarray(image[0, 0], np.float32)
    h0 = np.asarray(h[0, 0], np.float32)
    h1 = np.asarray(h[0, 1], np.float32)
    f = np.asarray(f_1[0, 0], np.float32)
    w0 = float(np.asarray(w_0)[0])

    logprob = -0.5 * I * I - LOG_SQRT_2PI
    Ud_full = (h0 - h1) * (-(w0 + logprob - f))
    hd = (h0 - h1).astype(np.float32)
    q0_full = (1.0 / (1.0 + np.exp(-hd))).astype(np.float32)
    G1 = _g1_full()
    CdBase_full = Ud_full - A1 * G1 - B1          # (D,H,W) fp32

    gsum = sum(math.exp(-(dz * dz + 1 + dx * dx) / S2S)
               for dz in (-1, 0, 1) for dx in (-1, 0, 1))
    gvec = np.zeros((H, 1), np.float32)
    gvec[0, 0] = gsum
    gvec[H - 1, 0] = gsum

    lng = np.zeros((H, 4), np.float32)
    for d2 in range(4):
        lng[:, d2] = -d2 / S2S

    bdvec = np.full((H, 1), -bd, np.float32)
    mats16 = _build_mats16(ad)

    in_maps = []
    for c in range(N_CORES):
        base = c * DL
        img_y0 = _vslab(I, base, -HALO, DL + HALO, np.float16)
        q0i = _vslab(q0_full, base, -HALO, DL + HALO, np.float16)
        mask = np.zeros((H, 4, WS), np.float16)
        if c > 0:
            mask[:, 0:2, CV:CV + W] = 1.0
        if c < N_CORES - 1:
            mask[:, 2:4, CV:CV + W] = 1.0
        # maskW1: sum of g_s over offsets with x+s outside the volume
        mw1 = np.zeros((H, DL, W), np.float32)
        for (dz, dy, dx) in OFFSETS:
            g = _gmat((dz, dy, dx))
            oob = np.zeros((H, DL, W), bool)
            for i in range(DL):
                if not (0 <= base + i + dz < D):
                    oob[:, i, :] = True
            if dy == -1:
                oob[0, :, :] = True
            elif dy == 1:
                oob[H - 1, :, :] = True
            if dx == -1:
                oob[:, :, 0] |= True
            elif dx == 1:
                oob[:, :, W - 1] |= True
            mw1 += g * oob
        in_maps.append({
            "mats16": mats16,
            "img_y0": img_y0,
            "img_ym": _yshift(img_y0, -1),
            "img_yp": _yshift(img_y0, +1),
            "img_x1": _xshift1(img_y0),
            "cdbase": np.ascontiguousarray(
                np.stack([CdBase_full[base + i] for i in range(DL)], 1)),
            "mw1e": (B1 * mw1).astype(np.float16),
            "q0i": q0i,
            "q1i": _xshift1(q0i),
            "mask": mask,
            "gvec": gvec,
            "lng": lng,
            "bdvec": bdvec,
        })
    return in_maps


# ---------------- bass program ----------------
def _build_nc(stage="full"):
    import concourse.bass as bass
    import concourse.tile as tile
    from concourse import bacc, mybir
    from contextlib import ExitStack

    dt16 = mybir.dt.float16
    dt32 = mybir.dt.float32
    AF = mybir.ActivationFunctionType
    OP = mybir.AluOpType

    nc = bacc.Bacc("TRN2", target_bir_lowering=False, debug=False,
                   num_devices=N_CORES)

    din = {}
    din["mats16"] = nc.dram_tensor("mats16", [N_MATS, H, H], dt16,
                                   kind="ExternalInput").ap()
    for nm in ("img_y0", "img_ym", "img_yp", "img_x1", "q0i", "q1i"):
        din[nm] = nc.dram_tensor(nm, [H, NP, WS], dt16,
                                 kind="ExternalInput").ap()
    din["cdbase"] = nc.dram_tensor("cdbase", [H, DL, W], dt32,
                                   kind="ExternalInput").ap()
    din["mw1e"] = nc.dram_tensor("mw1e", [H, DL, W], dt16,
                                 kind="ExternalInput").ap()
    din["mask"] = nc.dram_tensor("mask", [H, 4, WS], dt16,
                                 kind="ExternalInput").ap()
    din["gvec"] = nc.dram_tensor("gvec", [H, 1], dt32,
                                 kind="ExternalInput").ap()
    din["lng"] = nc.dram_tensor("lng", [H, 4], dt32,
                                kind="ExternalInput").ap()
    din["bdvec"] = nc.dram_tensor("bdvec", [H, 1], dt32,
                                  kind="ExternalInput").ap()
    h_out = nc.dram_tensor("h_out", [2, DL, H, W], dt32,
                           kind="ExternalOutput").ap()
    dbg = {}
    if DEBUG:
        for nm, shp, dt_ in [("dbg_rho", [H, 10, 132], dt16),
                             ("dbg_den", [H, DL, W], dt32),
                             ("dbg_rden", [H, DL, W], dt16),
                             ("dbg_cd", [H, DL, W], dt32),
                             ("dbg_diff1", [H, DL, W], dt32)]:
            dbg[nm] = nc.dram_tensor(nm, shp, dt_, kind="ExternalOutput").ap()

    with tile.TileContext(nc) as tc, ExitStack() as ctx:
        consts = ctx.enter_context(tc.tile_pool(name="consts", bufs=1))
        work = ctx.enter_context(tc.tile_pool(name="work", bufs=1))
        rhop = ctx.enter_context(tc.tile_pool(name="rhop", bufs=1))
        vpool = ctx.enter_context(tc.tile_pool(name="vpool", bufs=6))
        tmp = ctx.enter_context(tc.tile_pool(name="tmp", bufs=6))
        tmp2 = ctx.enter_context(tc.tile_pool(name="tmp2", bufs=2))
        psum = ctx.enter_context(
            tc.tile_pool(name="psum", bufs=2, space="PSUM"))
        psum_t1 = ctx.enter_context(
            tc.tile_pool(name="psum_t1", bufs=1, space="PSUM"))
        dram = ctx.enter_context(tc.tile_pool(name="dram", bufs=2,
                                              space="DRAM"))

        # ---- image slabs first: the rho chains depend on them ----
        imgs = ctx.enter_context(tc.tile_pool(name="imgs", bufs=1))
        img = {}
        for nm in ("img_y0", "img_x1", "img_ym", "img_yp"):
            t = imgs.tile([H, NP, WS], dt16, name=nm)
            nc.sync.dma_start(t[:], din[nm][:])
            img[nm] = t
        img_y = {-1: img["img_ym"], 0: img["img_y0"], 1: img["img_yp"]}

        # ---- constants ----
        lngt = consts.tile([H, 4], dt32)
        nc.sync.dma_start(lngt[:], din["lng"][:])
        gvect = consts.tile([H, 1], dt32)
        nc.sync.dma_start(gvect[:], din["gvec"][:])
        mats = consts.tile([H, N_MATS, H], dt16)
        for i in range(N_MATS):
            nc.sync.dma_start(mats[:, i, :], din["mats16"][i, :, :])
        bdvt = consts.tile([H, 1], dt32)
        nc.sync.dma_start(bdvt[:], din["bdvec"][:])
        cdb32 = consts.tile([H, DL, W], dt32)
        nc.sync.dma_start(cdb32[:], din["cdbase"][:])
        mw1e = consts.tile([H, DL, W], dt16)
        nc.sync.dma_start(mw1e[:], din["mw1e"][:])
        maskt = consts.tile([H, 4, WS], dt16)
        nc.sync.dma_start(maskt[:], din["mask"][:])

        def mat(i):
            return mats[:, i, :]

        # ---- persistent work tiles ----
        q0 = work.tile([H, NP, WS], dt16)
        q1 = work.tile([H, NP, WS], dt16)
        nc.sync.dma_start(q0[:], din["q0i"][:])
        nc.sync.dma_start(q1[:], din["q1i"][:])
        t1sb = work.tile([H, NP, W], dt16)
        cdhi = work.tile([H, DL, W], dt16)
        cdlo = work.tile([H, DL, W], dt16)
        rden = work.tile([H, DL, W], dt16)
        dtile = work.tile([H, DL, W], dt32)   # diff (combine output)

        # =========== PHASE 1: rho', den, rden, Cd ===========

        # rho tiles: mirror sources on 10 plane slots, others on 8
        rho = {}
        for s in COMPUTED:
            dz, dy, dx = s
            big = s in MSRC
            lo = 1 if big else 2    # first covered frame slot
            n = 10 if big else 8
            ia = img_y[dy][:, lo:lo + n, 2:134]
            if dx % 2 == 0:
                ib = img["img_y0"][:, lo + dz:lo + n + dz, 2 + dx:134 + dx]
            else:
                ib = img["img_x1"][:, lo + dz:lo + n + dz, 1 + dx:133 + dx]
            dlt = tmp.tile([H, 10, 132], dt16, tag="delta")
            nc.vector.tensor_tensor(dlt[:, 0:n, :], ia, ib, OP.subtract)
            sq = tmp.tile([H, 10, 132], dt16, tag="sq")
            nc.vector.tensor_tensor(sq[:, 0:n, :], dlt[:, 0:n, :],
                                    dlt[:, 0:n, :], OP.mult)
            r = rhop.tile([H, n, 132], dt16, name=f"rho_{s}")
            d2 = dz * dz + dy * dy + dx * dx
            nc.scalar.activation(r[:], sq[:, 0:n, :], AF.Exp,
                                 scale=-1.0 / S2R,
                                 bias=lngt[:, d2:d2 + 1])
            # (tile, own-plane index offset within tile, col shift)
            rho[s] = (r, 1 - lo, 0)
        for s in MIRRORED:
            dz, dy, dx = s
            src, po, _ = rho[(-dz, 0, -dx)]
            rho[s] = (src, po + dz, dx)

        # den psum: sum_s SD[dy] @ rho_s (own planes); rho_(0,0,0) == 1
        ones16 = work.tile([H, DL, W], dt16)
        nc.vector.memset(ones16[:], 1.0)
        ps_den = psum.tile([H, DL, W], dt32, tag="ps")
        for si, s in enumerate(OFFSETS):
            dz, dy, dx = s
            sd = {-1: M_SDM, 0: M_I, 1: M_SDP}[dy]
            for hf in range(2):
                if s == (0, 0, 0):
                    rhs = ones16[:, hf * 4:hf * 4 + 4, :]
                else:
                    r, po, px = rho[s]
                    rhs = r[:, 1 + po + hf * 4: 5 + po + hf * 4,
                            2 + px: 130 + px]
                nc.tensor.matmul(ps_den[:, hf * 4:hf * 4 + 4, :],
                                 mat(sd), rhs,
                                 start=(si == 0), stop=(si == 26))

        # E = exp(-I^2/S2R); den += gvec*E; rden = 1/den (fast recip + Newton)
        isq = tmp2.tile([H, DL, W], dt16, tag="scr8")
        nc.vector.tensor_tensor(isq[:],
                                img["img_y0"][:, 2:10, 4:132],
                                img["img_y0"][:, 2:10, 4:132], OP.mult)
        emap = work.tile([H, DL, W], dt16)
        nc.scalar.activation(emap[:], isq[:], AF.Exp, scale=-1.0 / S2R)
        # tA = (B1*maskW1)*E  (independent of den; hoisted off the tail)
        tA = tmp2.tile([H, DL, W], dt16, tag="scr8")
        nc.vector.tensor_tensor(tA[:], mw1e[:], emap[:], OP.mult)
        den_sb = work.tile([H, DL, W], dt32)
        nc.vector.scalar_tensor_tensor(den_sb[:], emap[:], gvect[:],
                                       ps_den[:], OP.mult, OP.add)
        r32 = tmp2.tile([H, DL, W], dt32, tag="u32")
        rscr = tmp2.tile([H, DL, W], dt32, tag="u32")
        nc.vector.reciprocal_approx_accurate(r32[:], den_sb[:], rscr[:])
        nc.scalar.copy(rden[:], r32[:])

        # Cd = CdBase + tA * rden ; split into fp16 hi+lo
        tB = tmp2.tile([H, DL, W], dt16, tag="scr8")
        nc.vector.tensor_tensor(tB[:], tA[:], rden[:], OP.mult)
        cd32 = work.tile([H, DL, W], dt32)
        nc.vector.tensor_tensor(cd32[:], tB[:], cdb32[:], OP.add)
        nc.vector.tensor_copy(cdhi[:], cd32[:])
        with nc.allow_low_precision(reason="hi/lo residual split"):
            nc.vector.scalar_tensor_tensor(cdlo[:], cdhi[:], -1.0,
                                           cd32[:], OP.mult, OP.add)

        if DEBUG:
            nc.sync.dma_start(dbg["dbg_rho"][:, 0:10, :],
                              rho[(0, 0, 1)][0][:])
            nc.sync.dma_start(dbg["dbg_den"][:], den_sb[:])
            nc.sync.dma_start(dbg["dbg_rden"][:], rden[:])
            nc.sync.dma_start(dbg["dbg_cd"][:], cd32[:])

        done = False
        if stage == "pre":
            dummy = work.tile([H, W], dt32)
            nc.vector.tensor_copy(dummy[:], cd32[:, 0, :])
            nc.sync.dma_start(h_out[0, 0, :, :], dummy[:])
            done = True

        # ======== one CRF message pass ========
        def message_pass(psA, psB, qa, qb):
            # psA = Cd (fp16 hi+lo) + blur terms (-ad folded in DH mats)
            for hf in range(2):
                nc.tensor.matmul(psA[:, hf * 4:hf * 4 + 4, :], mat(M_I),
                                 cdhi[:, hf * 4:hf * 4 + 4, :],
                                 start=True, stop=False)
                nc.tensor.matmul(psA[:, hf * 4:hf * 4 + 4, :], mat(M_I),
                                 cdlo[:, hf * 4:hf * 4 + 4, :],
                                 start=False, stop=False)
            t1ps = psum_t1.tile([H, NP, W], dt32, tag="t1")
            for ti, dx in enumerate(range(-2, 3)):
                for third in range(3):
                    sl = slice(third * 4, third * 4 + 4)
                    if dx % 2 == 0:
                        rhs = qa[:, sl, CV + dx:CV + W + dx]
                    else:
                        rhs = qb[:, sl, CV - 1 + dx:CV - 1 + W + dx]
                    nc.tensor.matmul(t1ps[:, sl, :], mat(M_CI + ti), rhs,
                                     start=(ti == 0), stop=(ti == 4))
            nc.scalar.copy(t1sb[:], t1ps[:])
            for ti, dz in enumerate(range(-2, 3)):
                for hf in range(2):
                    rhs = t1sb[:, 2 + hf * 4 + dz:6 + hf * 4 + dz, :]
                    nc.tensor.matmul(psA[:, hf * 4:hf * 4 + 4, :],
                                     mat(M_DH + ti), rhs,
                                     start=False, stop=(ti == 4 and hf == 1))
            # psB = raw bilateral sum (products on DVE/GPSIMD, adds on PE)
            # halo-independent (dz=0) and q0-only (even dx) products first:
            # they can run while the halo AllGather and q1 sigmoid complete
            ordered = sorted(OFFSETS, key=lambda s: (s[0] != 0, s[2] % 2 != 0))
            for si, s in enumerate(ordered):
                dz, dy, dx = s
                sd = {-1: M_SDM, 0: M_I, 1: M_SDP}[dy]
                if s == (0, 0, 0):
                    # rho == 1: PE reads q directly, no product needed
                    for hf in range(2):
                        nc.tensor.matmul(
                            psB[:, hf * 4:hf * 4 + 4, :], mat(sd),
                            qa[:, 2 + hf * 4:6 + hf * 4, CV:CV + W],
                            start=(si == 0), stop=(si == 26 and hf == 1))
                    continue
                r, po, px = rho[s]
                if dx % 2 == 0:
                    qr = qa[:, 2 + dz:10 + dz, CV + dx:CV + W + dx]
                else:
                    qr = qb[:, 2 + dz:10 + dz, CV - 1 + dx:CV - 1 + W + dx]
                rr = r[:, 1 + po:9 + po, 2 + px:130 + px]
                v = vpool.tile([H, DL, W], dt16, tag="v")
                nc.vector.tensor_tensor(v[:], qr, rr, OP.mult)
                for hf in range(2):
                    nc.tensor.matmul(psB[:, hf * 4:hf * 4 + 4, :], mat(sd),
                                     v[:, hf * 4:hf * 4 + 4, :],
                                     start=(si == 0), stop=(si == 26 and
                                                            hf == 1))

        # =========== PHASE 3: iterations ===========
        if done:
            n_iters = 0
        else:
            if stage.startswith("it"):
                n_iters = int(stage[2:])
            elif stage.startswith("noex") and len(stage) > 4:
                n_iters = int(stage[4:])
            elif stage.startswith("nag"):
                n_iters = int(stage[3:])
            else:
                n_iters = NUM_ITERS
            rank = nc.partition_id()
            lo_r = (rank + N_CORES - 1) % N_CORES
            hi_r = (rank + 1) % N_CORES

        for it in range(n_iters):
            if it > 0:
                for hf in range(2):
                    sl = slice(2 + hf * 4, 6 + hf * 4)
                    dsl = dtile[:, hf * 4:hf * 4 + 4, :]
                    nc.scalar.activation(q0[:, sl, CV:CV + W], dsl,
                                         AF.Sigmoid)
                    nc.scalar.activation(q1[:, sl, CV - 1:CV - 1 + W], dsl,
                                         AF.Sigmoid)
                if not stage.startswith("noex"):
                    cc_in = dram.tile([H, 4, WS], dt16, tag="cc_in")
                    cc_out = dram.tile([N_CORES, H, 4, WS], dt16,
                                       addr_space="Shared", tag="cc_out")
                    nc.sync.dma_start(cc_in[:, 0:2, :], q0[:, 2:4, :])
                    nc.sync.dma_start(cc_in[:, 2:4, :], q0[:, 8:10, :])
                    if not stage.startswith("nag"):
                        nc.gpsimd.collective_compute(
                            "AllGather", OP.bypass,
                            replica_groups=[list(range(N_CORES))],
                            ins=[cc_in[:].opt()], outs=[cc_out[:].opt()])
                    stg = tmp2.tile([H, 4, WS], dt16, tag="stage")
                    nc.sync.dma_start(stg[:, 0:2, :],
                                      cc_out[bass.ds(lo_r, 1), :, 2:4, :])
                    nc.sync.dma_start(stg[:, 2:4, :],
                                      cc_out[bass.ds(hi_r, 1), :, 0:2, :])
                    nc.vector.tensor_tensor(q0[:, 0:2, :], stg[:, 0:2, :],
                                            maskt[:, 0:2, :], OP.mult)
                    nc.vector.tensor_tensor(q0[:, 10:12, :], stg[:, 2:4, :],
                                            maskt[:, 2:4, :], OP.mult)
                    nc.vector.tensor_tensor(q1[:, 0:2, 0:WS - 1],
                                            stg[:, 0:2, 1:WS],
                                            maskt[:, 0:2, 1:WS], OP.mult)
                    nc.vector.tensor_tensor(q1[:, 10:12, 0:WS - 1],
                                            stg[:, 2:4, 1:WS],
                                            maskt[:, 2:4, 1:WS], OP.mult)
            psA = psum.tile([H, DL, W], dt32, tag="ps")
            psB = psum.tile([H, DL, W], dt32, tag="ps")
            message_pass(psA, psB, q0, q1)
            # combine per half: dtile = psA + (-bd)*rden*psB
            for hf in range(2):
                hsl = slice(hf * 4, hf * 4 + 4)
                u32 = tmp2.tile([H, 4, W], dt32, tag="u32h")
                nc.vector.scalar_tensor_tensor(u32[:], psB[:, hsl, :],
                                               bdvt[:], rden[:, hsl, :],
                                               OP.mult, OP.mult)
                nc.vector.tensor_tensor(dtile[:, hsl, :], psA[:, hsl, :],
                                        u32[:], OP.add)
            if DEBUG and it == 0:
                nc.sync.dma_start(dbg["dbg_diff1"][:], dtile[:])

        # =========== output ===========
        if not done:
            h0t = work.tile([H, DL, W], dt32)
            nc.scalar.activation(h0t[:], dtile[:], AF.Sigmoid)
            h1t = work.tile([H, DL, W], dt32)
            nc.vector.tensor_scalar(h1t[:], h0t[:], -1.0, 1.0, OP.mult,
                                    OP.add)
            for d_ in range(DL):
                nc.sync.dma_start(h_out[0, d_, :, :], h0t[:, d_, :])
                nc.sync.dma_start(h_out[1, d_, :, :], h1t[:, d_, :])

    nc.compile()
    return nc


def _get_nc(stage="full"):
    key = f"nc_{stage}"
    if key not in _CACHE:
        _CACHE[key] = _build_nc(stage)
    return _CACHE[key]


def kernel(image, h, f_1, w_0, spatial_ker_weights, bilateral_ker_weights,
           compatibility_matrix):
    from concourse.bass_utils import run_bass_kernel_spmd

    A1, B1, ad, bd = _host_scalars(
        np.asarray(spatial_ker_weights, np.float64),
        np.asarray(bilateral_ker_weights, np.float64),
        np.asarray(compatibility_matrix, np.float64))
    in_maps = _build_core_inputs(np.asarray(image), np.asarray(h),
                                 np.asarray(f_1), np.asarray(w_0),
                                 A1, B1, ad, bd)
    nc = _get_nc()
    res = run_bass_kernel_spmd(nc, in_maps, list(range(N_CORES)))
    _CACHE["last_results"] = res

    h_new = np.empty((1, 2, D, H, W), np.float32)
    for c in range(N_CORES):
        out = res.results[c]["h_out"]          # [2, DL, H, W]
        h_new[0, :, c * DL:(c + 1) * DL] = out
    return (h_new, np.asarray(f_1))
